# revision 9
# baseline (speedup 1.0000x reference)
"""CfC RNN decoder (3-layer NCP) on 8 TRN2 NeuronCores, data-parallel over batch.

Layout strategy (per core, B_loc=64):
- Batch-major matmuls: out z[batch(64 part), units] = xc @ W.T, with the
  transposed activation xcT as the stationary operand and the pre-transposed
  weights [K, 3n] (gates ff1|ff2|t folded, Wt = Wta+Wtb) as the moving operand.
- float32r matmuls (1 cyc/row at N>=256), fp32 PSUM accumulate.
- Col-tiling packs pairs of 64-wide matmuls into the 128-wide PE array:
  ff1 -> psum[0:64], ff2 -> psum[64:128]; the t gate is split into two
  unit-chunks occupying the two partition halves.
- Recurrent state is kept TRANSPOSED (hT, [units-on-partitions, 64]) in
  ping-pong buffers; produced each step by PE transposes into persistent
  zero-initialized PSUM tiles, then one ACT copy per group (also casts to
  float32r and maintains zero padding rows).
"""
import os
import numpy as np

import concourse.bass as bass
import concourse.tile as tile
import concourse.mybir as mybir
from concourse.bass_utils import run_bass_kernel_spmd

import concourse.mybir as _mybir

F32 = mybir.dt.float32
F32R = mybir.dt.float32r
BF16 = mybir.dt.bfloat16
AF = mybir.ActivationFunctionType

N_CORES = 8
B, T, I_DIM = 512, 128, 512
N = [538, 358, 128]
B_LOC = B // N_CORES  # 64

# K-tile layouts (rows of the pre-transposed weight matrices)
# L0: x rows 0:512 (tiles 0-3), h0 rows 512:1050, pad to 1152 (tiles 4-8)
# L1: h0 rows 0:538 pad to 640 (tiles 0-4), h1 rows 640:998 pad to 1024 (5-7)
# L2: h1 rows 0:358 pad to 384 (tiles 0-2), h2 rows 384:512 (tile 3)
K0, K1, K2 = 1152, 1024, 512
KT0, KT1, KT2 = 9, 8, 4

LAST_RESULTS = [None]  # stashed BassKernelResults for test harness


# ---------------------------------------------------------------- host packing

def _pack_weights(mask, w1, w2, wa, wb, xin_dim, n, krows, hoff):
    """Build [K, 3n] moving-operand matrix: cols [G1 | G2 | Gt], rows
    [xin part | pad | h part | pad] with host-side masking and t-gate fold."""
    g1 = (w1 * mask).astype(np.float32)
    g2 = (w2 * mask).astype(np.float32)
    gt = (wa + wb).astype(np.float32)
    out = np.zeros((krows, 3 * n), dtype=np.float32)
    for gi, g in enumerate((g1, g2, gt)):
        out[:xin_dim, gi * n:(gi + 1) * n] = g[:, :xin_dim].T
        out[hoff:hoff + n_h(g, xin_dim), gi * n:(gi + 1) * n] = g[:, xin_dim:].T
    return out


def n_h(g, xin_dim):
    return g.shape[1] - xin_dim


def _pack_all(inputs):
    w0 = _pack_weights(inputs["mask0"], inputs["Wff1_0"], inputs["Wff2_0"],
                       inputs["Wta_0"], inputs["Wtb_0"], 512, 538, K0, 512)
    w1 = _pack_weights(inputs["mask1"], inputs["Wff1_1"], inputs["Wff2_1"],
                       inputs["Wta_1"], inputs["Wtb_1"], 538, 358, K1, 640)
    w2raw = _pack_weights(inputs["mask2"], inputs["Wff1_2"], inputs["Wff2_2"],
                          inputs["Wta_2"], inputs["Wtb_2"], 358, 128, K2, 384)
    # L2 moving layout: [G1(128) | G2(128) | Gt(128) | zeros(128)] = 512 cols
    w2 = np.zeros((K2, 512), dtype=np.float32)
    w2[:, 0:384] = w2raw
    wfc = np.zeros((128, 256), dtype=np.float32)
    wfc[:, 0:128] = inputs["Wfc"].T.astype(np.float32)
    return w0, w1, w2, wfc


def _pack_biases(inputs):
    """Returns per-layer replicated bias tiles (or None when all-zero)."""
    out = {}
    b1 = [inputs[f"bff1_{l}"] for l in range(3)]
    b2 = [inputs[f"bff2_{l}"] for l in range(3)]
    bt = [(inputs[f"bta_{l}"] + inputs[f"btb_{l}"]) for l in range(3)]
    anyb = any(np.any(np.asarray(x)) for x in b1 + b2 + bt) or np.any(
        np.asarray(inputs["bfc"]))
    out["enable"] = bool(anyb)
    if not anyb:
        return out
    # L0 ff: (128, 538): rows 0:64 b1, 64:128 b2 ; t: (128,269): halves of bt
    def rep(v, rows=64):
        return np.broadcast_to(np.asarray(v, np.float32), (rows, v.shape[0]))
    out["bff0"] = np.concatenate([rep(b1[0]), rep(b2[0])], 0).copy()
    out["bt0"] = np.concatenate([rep(bt[0][:269]), rep(bt[0][269:538])], 0).copy()
    out["bff1"] = np.concatenate([rep(b1[1]), rep(b2[1])], 0).copy()
    out["bt1"] = rep(bt[1]).copy()
    bl2 = np.zeros((128, 256), np.float32)
    bl2[0:64, 0:128] = rep(b1[2])
    bl2[0:64, 128:256] = rep(b2[2])
    bl2[64:128, 0:128] = rep(bt[2])
    out["bl2"] = bl2
    out["bfc"] = rep(inputs["bfc"]).copy()
    return out


# ---------------------------------------------------------------- wait fixing

_uid = [0]


def _split_excess_waits(nc, limit=1):
    """This walrus build accepts at most ONE sync wait per instruction.
    Move excess waits onto inserted same-engine NOPs placed just before."""
    for fn in nc.m.functions:
        for bb in fn.blocks:
            new_insts = []
            for inst in bb.instructions:
                si = inst.sync_info
                waits = list(si.on_wait) if si is not None and si.on_wait else []
                if len(waits) > limit:
                    keep = waits[:limit]
                    for w in waits[limit:]:
                        _uid[0] += 1
                        nop = _mybir.InstNoOp(
                            name=f"wsplit_{_uid[0]}", engine=inst.engine,
                            ins=[], outs=[], text_hint="wait_split")
                        nop.sync_info = _mybir.SyncInfo(on_wait=[w], on_update=[])
                        nc.register_instruction(nop, overwrite=True)
                        new_insts.append(nop)
                    inst.sync_info = _mybir.SyncInfo(
                        on_wait=keep,
                        on_update=list(si.on_update) if si else [])
                new_insts.append(inst)
            bb.instructions[:] = new_insts


# ---------------------------------------------------------------- bass builder

_NC_CACHE = {}


def build_bass(t_steps=T, with_bias=False):
    key = (t_steps, with_bias)
    if key in _NC_CACHE:
        return _NC_CACHE[key]

    nc = bass.Bass("TRN2", target_bir_lowering=False, debug=False)

    xT = nc.dram_tensor("xT", [t_steps, 512, B_LOC], BF16, kind="ExternalInput").ap()
    w0 = nc.dram_tensor("w0", [K0, 1614], BF16, kind="ExternalInput").ap()
    w1 = nc.dram_tensor("w1", [K1, 1074], BF16, kind="ExternalInput").ap()
    w2 = nc.dram_tensor("w2", [K2, 512], BF16, kind="ExternalInput").ap()
    wfc = nc.dram_tensor("wfc", [128, 256], BF16, kind="ExternalInput").ap()
    h0T_d = nc.dram_tensor("h0T", [640, B_LOC], BF16, kind="ExternalInput").ap()
    h1T_d = nc.dram_tensor("h1T", [384, B_LOC], BF16, kind="ExternalInput").ap()
    h2T_d = nc.dram_tensor("h2T", [128, B_LOC], BF16, kind="ExternalInput").ap()
    ident_d = nc.dram_tensor("ident", [64, 64], F32, kind="ExternalInput").ap()
    bias_d = {}
    if with_bias:
        for nm, shp in (("bff0", (128, 538)), ("bt0", (128, 269)),
                        ("bff1", (128, 358)), ("bt1", (64, 358)),
                        ("bl2", (128, 256)), ("bfc", (64, 128))):
            bias_d[nm] = nc.dram_tensor(
                "b_" + nm, list(shp), F32, kind="ExternalInput").ap()

    preds = nc.dram_tensor("preds", [t_steps, B_LOC, 128], F32,
                           kind="ExternalOutput").ap()
    hn = nc.dram_tensor("hn", [B_LOC, 1024], F32, kind="ExternalOutput").ap()

    from contextlib import ExitStack
    with tile.TileContext(nc, trace_sim=False) as tc, ExitStack() as ctx:
        _build_body(ctx, nc, tc, t_steps, xT, w0, w1, w2, wfc,
                    h0T_d, h1T_d, h2T_d, ident_d, bias_d, preds, hn)

    _split_excess_waits(nc, limit=1)
    _NC_CACHE[key] = nc
    return nc


def _build_body(ctx, nc, tc, t_steps, xT, w0d, w1d, w2d, wfcd,
                h0T_d, h1T_d, h2T_d, ident_d, bias_d, preds, hn):
    with_bias = bool(bias_d)
    cpool = ctx.enter_context(tc.tile_pool(name="consts", bufs=1))
    spool = ctx.enter_context(tc.tile_pool(name="state", bufs=1))
    apool = ctx.enter_context(tc.tile_pool(name="acts", bufs=3))
    hpool = ctx.enter_context(tc.tile_pool(name="hnew", bufs=2))
    ppf = ctx.enter_context(tc.tile_pool(name="ppf", bufs=3, space="PSUM"))
    ppt = ctx.enter_context(tc.tile_pool(name="ppt", bufs=2, space="PSUM"))
    pptr = ctx.enter_context(tc.tile_pool(name="pptr", bufs=1, space="PSUM"))
    ppfc = ctx.enter_context(tc.tile_pool(name="ppfc", bufs=1, space="PSUM"))

    # ---- constants
    w0t = cpool.tile([128, KT0, 1614], BF16, tag="w0")
    w1t = cpool.tile([128, KT1, 1074], BF16, tag="w1")
    w2t = cpool.tile([128, KT2, 512], BF16, tag="w2")
    wfct = cpool.tile([128, 256], BF16, tag="wfc")
    ident = cpool.tile([64, 64], F32, tag="ident")
    nc.sync.dma_start(w0t[:], w0d.rearrange("(k p) n -> p k n", p=128))
    nc.sync.dma_start(w1t[:], w1d.rearrange("(k p) n -> p k n", p=128))
    nc.sync.dma_start(w2t[:], w2d.rearrange("(k p) n -> p k n", p=128))
    nc.sync.dma_start(wfct[:], wfcd)
    nc.sync.dma_start(ident[:], ident_d)
    btiles = {}
    for nm, ap in bias_d.items():
        bt_ = cpool.tile(list(ap.shape), F32, tag="b_" + nm)
        nc.sync.dma_start(bt_[:], ap)
        btiles[nm] = bt_

    # ---- state (ping-pong)
    xbuf = [spool.tile([128, 4, B_LOC], BF16, tag=f"xbuf{i}", name=f"xbuf{i}")
            for i in range(2)]
    xc1 = [spool.tile([128, 5, B_LOC], BF16, tag=f"xc1_{i}", name=f"xc1_{i}")
           for i in range(2)]
    xc2 = [spool.tile([128, 4, B_LOC], BF16, tag=f"xc2_{i}", name=f"xc2_{i}")
           for i in range(2)]
    nc.sync.dma_start(xc1[1][:], h0T_d.rearrange("(k p) b -> p k b", p=128))
    nc.sync.dma_start(xc2[1][:, 0:3, :], h1T_d.rearrange("(k p) b -> p k b", p=128))
    nc.sync.dma_start(xc2[0][:, 3, :], h2T_d)
    nc.sync.dma_start(xbuf[0][:], xT[0].rearrange("(k p) b -> p k b", p=128))

    # ---- persistent transpose psum tiles, zeroed once (pads stay zero)
    tr0 = pptr.tile([128, 320], F32, tag="tr0")   # h0T chunks (5x64)
    tr1 = pptr.tile([128, 256], F32, tag="tr1")   # h1T (3x64) + h2T (64)
    nc.vector.memset(tr0[:], 0.0)
    nc.vector.memset(tr1[:], 0.0)

    h_last = [None, None, None]

    for t in range(t_steps):
        par, nxt = t % 2, (t + 1) % 2

        if t + 1 < t_steps:
            nc.sync.dma_start(
                xbuf[nxt][:], xT[t + 1].rearrange("(k p) b -> p k b", p=128))

        def lhs0(kt):
            return xbuf[par][:, kt, :] if kt < 4 else xc1[nxt][:, kt - 4, :]

        # ---------- layer 0 matmuls ----------
        pf0 = ppf.tile([128, 269], F32, tag="pf")
        pf1 = ppf.tile([128, 269], F32, tag="pf")
        pt0 = ppt.tile([128, 269], F32, tag="pt")
        for c, pf in ((0, pf0), (1, pf1)):
            for j in range(KT0):
                lt = lhs0(j)
                st, sp = j == 0, j == KT0 - 1
                nc.tensor.matmul(pf[0:64, :], lt,
                                 w0t[:, j, 269 * c:269 * c + 269],
                                 start=st, stop=sp)
                nc.tensor.matmul(pf[64:128, :], lt,
                                 w0t[:, j, 538 + 269 * c:538 + 269 * c + 269],
                                 start=st, stop=sp)
        for j in range(KT0):
            lt = lhs0(j)
            st, sp = j == 0, j == KT0 - 1
            nc.tensor.matmul(pt0[0:64, :], lt, w0t[:, j, 1076:1345],
                             start=st, stop=sp)
            nc.tensor.matmul(pt0[64:128, :], lt, w0t[:, j, 1345:1614],
                             start=st, stop=sp)

        # ---------- layer 1 matmuls, h1-part first (data from t-1) ----------
        pfL1 = ppf.tile([128, 358], F32, tag="pf")
        ptL1 = ppt.tile([64, 358], F32, tag="pt")
        kts1 = [5, 6, 7, 0, 1, 2, 3, 4]

        def lhs1(kt):
            return xc1[par][:, kt, :] if kt < 5 else xc2[nxt][:, kt - 5, :]

        def l1_seg(kt_list, first, last):
            for j in kt_list:
                lt = lhs1(j)
                st, sp = j == first, j == last
                nc.tensor.matmul(pfL1[0:64, :], lt, w1t[:, j, 0:358],
                                 start=st, stop=sp)
                nc.tensor.matmul(pfL1[64:128, :], lt, w1t[:, j, 358:716],
                                 start=st, stop=sp)
                nc.tensor.matmul(ptL1[:], lt, w1t[:, j, 716:1074],
                                 start=st, stop=sp)

        l1_seg([5, 6, 7], 5, 4)

        # ---------- layer 0 activations + combine + transpose ----------
        if with_bias:
            nc.vector.tensor_add(pf0[:], pf0[:], btiles["bff0"][:, 0:269])
            nc.vector.tensor_add(pf1[:], pf1[:], btiles["bff0"][:, 269:538])
            nc.vector.tensor_add(pt0[:], pt0[:], btiles["bt0"][:])
        h0n = hpool.tile([64, 538], F32, tag="h0n")
        for c, pf in ((0, pf0), (1, pf1)):
            t1 = apool.tile([64, 269], F32, tag="A", bufs=6)
            t2 = apool.tile([64, 269], F32, tag="A", bufs=6)
            sg = apool.tile([64, 269], F32, tag="S", bufs=4)
            nc.scalar.activation(t1[:], pf[0:64, :], AF.Tanh)
            nc.scalar.activation(t2[:], pf[64:128, :], AF.Tanh)
            nc.scalar.activation(sg[:], pt0[64 * c:64 * c + 64, :], AF.Sigmoid)
            tmp = apool.tile([64, 269], F32, tag="tmp", bufs=3)
            nc.vector.tensor_sub(tmp[:], t2[:], t1[:])
            nc.vector.tensor_mul(tmp[:], tmp[:], sg[:])
            nc.vector.tensor_add(h0n[:, 269 * c:269 * c + 269],
                                 t1[:], tmp[:])
        # h0T transposes -> tr0, one copy into xc1[par] (casts to f32r)
        for c in range(5):
            w = min(128, 538 - 128 * c)
            nc.tensor.transpose(tr0[0:w, 64 * c:64 * c + 64],
                                h0n[:, 128 * c:128 * c + w], ident[:])
        nc.scalar.copy(xc1[par][:, 0:5, :],
                       tr0[:].rearrange("p (s b) -> p s b", b=64))

        # ---------- layer 1 rest ----------
        l1_seg([0, 1, 2, 3, 4], 5, 4)
        if with_bias:
            nc.vector.tensor_add(pfL1[:], pfL1[:], btiles["bff1"][:])
            nc.vector.tensor_add(ptL1[:], ptL1[:], btiles["bt1"][:])
        u1 = apool.tile([64, 358], F32, tag="A", bufs=6)
        u2 = apool.tile([64, 358], F32, tag="A", bufs=6)
        SL1 = apool.tile([64, 358], F32, tag="S", bufs=4)
        nc.scalar.activation(u1[:], pfL1[0:64, :], AF.Tanh)
        nc.scalar.activation(u2[:], pfL1[64:128, :], AF.Tanh)
        nc.scalar.activation(SL1[:], ptL1[:], AF.Sigmoid)
        h1n = hpool.tile([64, 358], F32, tag="h1n")
        tmp1 = apool.tile([64, 358], F32, tag="tmp", bufs=3)
        nc.vector.tensor_sub(tmp1[:], u2[:], u1[:])
        nc.vector.tensor_mul(tmp1[:], tmp1[:], SL1[:])
        nc.vector.tensor_add(h1n[:], u1[:], tmp1[:])
        for c in range(3):
            w = min(128, 358 - 128 * c)
            nc.tensor.transpose(tr1[0:w, 64 * c:64 * c + 64],
                                h1n[:, 128 * c:128 * c + w], ident[:])
        nc.scalar.copy(xc2[par][:, 0:3, :],
                       tr1[:, 0:192].rearrange("p (s b) -> p s b", b=64))

        # ---------- layer 2 ----------
        pL2 = ppf.tile([128, 256], F32, tag="pf")
        for j in (3, 0, 1, 2):
            lt = xc2[par][:, j, :]
            st, sp = j == 3, j == 2
            nc.tensor.matmul(pL2[0:64, :], lt, w2t[:, j, 0:256],
                             start=st, stop=sp)
            nc.tensor.matmul(pL2[64:128, :], lt, w2t[:, j, 256:512],
                             start=st, stop=sp)
        if with_bias:
            nc.vector.tensor_add(pL2[:], pL2[:], btiles["bl2"][:])
        AL2 = apool.tile([64, 256], F32, tag="A", bufs=6)
        SL2 = apool.tile([64, 128], F32, tag="S", bufs=4)
        nc.scalar.activation(AL2[:], pL2[0:64, :], AF.Tanh)
        nc.scalar.activation(SL2[:], pL2[64:128, 0:128], AF.Sigmoid)
        h2n = hpool.tile([64, 128], F32, tag="h2n")
        tmp2 = apool.tile([64, 128], F32, tag="tmp")
        nc.vector.tensor_sub(tmp2[:], AL2[:, 128:256], AL2[:, 0:128])
        nc.vector.tensor_mul(tmp2[:], tmp2[:], SL2[:])
        nc.vector.tensor_add(h2n[:], AL2[:, 0:128], tmp2[:])
        nc.tensor.transpose(tr1[0:128, 192:256], h2n[:], ident[:])
        nc.scalar.copy(xc2[nxt][:, 3, :], tr1[:, 192:256])

        # ---------- final linear ----------
        pfc = ppfc.tile([64, 256], F32, tag="pfc")
        nc.tensor.matmul(pfc[:], xc2[nxt][:, 3, :], wfct[:],
                         start=True, stop=True)
        stage = apool.tile([64, 128], F32, tag="stage")
        if with_bias:
            nc.vector.tensor_add(stage[:], pfc[0:64, 0:128], btiles["bfc"][:])
        else:
            nc.scalar.copy(stage[:], pfc[0:64, 0:128])
        nc.sync.dma_start(preds[t], stage[:])

        h_last = [h0n, h1n, h2n]

    nc.sync.dma_start(hn[:, 0:538], h_last[0][:])
    nc.sync.dma_start(hn[:, 538:896], h_last[1][:])
    nc.sync.dma_start(hn[:, 896:1024], h_last[2][:])


# ---------------------------------------------------------------- entry point

def kernel(**inputs):
    inputs = {k: np.asarray(v) for k, v in inputs.items()}
    x = inputs["x"].astype(np.float32)
    h0 = inputs["h0"].astype(np.float32)
    t_steps = x.shape[1]

    w0, w1, w2, wfc = _pack_all(inputs)
    biases = _pack_biases(inputs)
    with_bias = biases.pop("enable")

    nc = build_bass(t_steps=t_steps, with_bias=with_bias)

    import ml_dtypes
    bf = ml_dtypes.bfloat16
    ident = np.eye(64, dtype=np.float32)
    w0b, w1b, w2b, wfcb = (a.astype(bf) for a in (w0, w1, w2, wfc))
    in_maps = []
    for c in range(N_CORES):
        rows = slice(c * B_LOC, (c + 1) * B_LOC)
        xc = np.ascontiguousarray(x[rows].transpose(1, 2, 0)).astype(bf)
        h0c = h0[rows]
        h0T = np.zeros((640, B_LOC), np.float32)
        h0T[0:538] = h0c[:, 0:538].T
        h1T = np.zeros((384, B_LOC), np.float32)
        h1T[0:358] = h0c[:, 538:896].T
        h2T = np.ascontiguousarray(h0c[:, 896:1024].T)
        m = {"xT": xc, "w0": w0b, "w1": w1b, "w2": w2b, "wfc": wfcb,
             "h0T": h0T.astype(bf), "h1T": h1T.astype(bf),
             "h2T": h2T.astype(bf), "ident": ident}
        if with_bias:
            for nm in ("bff0", "bt0", "bff1", "bt1", "bl2", "bfc"):
                m["b_" + nm] = biases[nm]
        in_maps.append(m)

    trace = bool(os.environ.get("KERNEL_TRACE"))
    res = run_bass_kernel_spmd(nc, in_maps, list(range(N_CORES)), trace=trace)
    LAST_RESULTS[0] = res

    preds_full = np.empty((B, t_steps, 128), np.float32)
    hn_full = np.empty((B, 1024), np.float32)
    for c in range(N_CORES):
        rows = slice(c * B_LOC, (c + 1) * B_LOC)
        preds_full[rows] = res.results[c]["preds"].transpose(1, 0, 2)
        hn_full[rows] = res.results[c]["hn"]
    return preds_full, hn_full
